# revision 72
# baseline (speedup 1.0000x reference)
"""Trainium2 Bass kernel for nn_NeuralAttention (MLP-scored attention).

Math (per head h, batch 1, n=512, dh=64, P=32):
  qkv = x @ Wqkv^T, split 'b n (d k h) -> k b h n d'
  qp = q@Wq^T+bq ; kp = k@Wk^T+bk
  a  = qp@W1q^T  ; c = kp@W1k^T          (W1 = [W1q | W1k])
  h1 = relu(a_i + c_j + b1)              # [n, n, 32]
  h2 = relu(h1 @ W2^T + b2)              # [n, n, 16]
  s  = h2 @ W3^T (+ b3, drops in softmax)
  attn = softmax(causal(s)) ; out = attn @ v ; y = out @ Wout^T

The scores are numerically near-uniform (std ~1e-4), so the whole
score path tolerates fp8.  The q/k projections are folded into the
a/c weights (a = x @ (W1q Wq Wqkv_q)^T) and computed as fp8
DoubleRow matmuls (K=256, 0.5 cyc/col), as are stage-2 (for a
tunable fraction of pairs) and stage-3.  The v / attn@v / Wout path
stays bf16 (errors there hit the output directly).  Scaling: h1 x16
(folded into a/c weights + s1const), W2 x4 (h2 x64), W3 x16; the
exp un-scales by 1/1024 via the activation scale argument.

Sharding: 16 heads over 8 cores (2 heads/core), Wout row-parallel;
host sums the 8 partial bf16 [1024, 512] outputs and transposes.

Layout ("j on partitions"): scores^T[j, i] in j-tiles of 128; pair
groups (m0, m0+1) share causal column offset 8*m0.  Stage-1
relu(a_i + c_j) is spread over DVE (bf16 4x / fp8 2x), GPSIMD and
ACT; stage-2.5 relu -> fp8 h2 over ACT/DVE (GPSIMD cannot read
PSUM).  DoubleRow dst must start at partition 0, so stage-3 (and
the c-bias matmul) use sliding zero-padded weight windows that park
each group's 16 (or 32) live rows at the right output partitions.
Scores PSUM is pre-initialized by an identity matmul with the
causal -1e30 mask; softmax denominator comes free from a
ones-column in the attn@v matmul.

Modeled (TimelineSim) per-core time: ~68.1 us (baseline 77.7);
measured rel. error vs fp32 reference: 4.3e-3.
"""

import sys

sys.path.insert(0, "/opt/trn_rl_repo")

from contextlib import ExitStack

import ml_dtypes
import numpy as np

import concourse.bass as bass
import concourse.tile as tile
from concourse import bacc, mybir
from concourse.bass_utils import run_bass_kernel_spmd

F32 = mybir.dt.float32
BF16 = mybir.dt.bfloat16
FP8 = mybir.dt.float8e4
U8 = mybir.dt.uint8
AF = mybir.ActivationFunctionType
ALU = mybir.AluOpType
PM = mybir.MatmulPerfMode

B, N, DIM = 1, 512, 1024
HEADS, DH = 16, 64
P, P2 = 32, 16
N_CORES = 8
HPC = HEADS // N_CORES  # heads per core = 2

S1 = 16.0   # h1 scale (folded into a/c weights and s1const)
S2 = 4.0    # extra W2 scale (h2 scale = S1*S2 = 64)
S3 = 16.0   # W3 scale; total score scale S1*S2*S3 = 1024

# scheduling tunables
TUNE = dict(
    fp8_num=7, fp8_den=20,   # fraction of large pairs with fp8 stage-2
    fp8_wmin=200,            # pairs narrower than this stay bf16
    fp8l_num=0, fp8l_den=20, # same for t==2 tiles (0 = off)
    fp8l_wmin=100,
    pair_ph=0,               # fp8 Bresenham phase offset
    s1f_pat="p",             # engine rotation for fp8 stage-1 (d/p/a)
    s1b_pat="d",        # engine rotation for bf16 stage-1
    s25_pat="aaad",          # engine rotation for stage-2.5 (no "p": PSUM)
    s25l_pat="aada",           # stage-2.5 rotation for late (small) tiles
    cmax=256,                # psum column chunk for stage-2/3
    s2_bufs=4,               # stage-2 psum tiles
    h1_bufs=36,              # stage-1 sbuf tiles
    h2_bufs=12,              # stage-2.5 sbuf tiles
    ex_bufs=5,               # exp sbuf tiles
    sc_bufs=2,               # scores psum tiles (shared tag, both heads)
    torder=(0, 1, 2, 3),     # j-tile processing order
    op_bufs=1,               # out' accumulator psum tiles (per head tag)
)
NT = N // 128           # j tiles = 4
KT = DIM // 128         # contraction tiles for projections = 8

# byte offsets in the packed constants blob (all 4-byte aligned)
_CO_S1C = 0            # f32 [128, 1]
_CO_B2R = 4            # f32 [128, 1]
_CO_W2B = 8            # bf16 [128, 64]
_CO_IDEN = 136         # bf16 [128, 128]
_CO_TRI = 392          # bf16 [128, 512]
_CO_W2B8 = 1416        # fp8 [128, 2, 128]
_CO_W3DR = 1672        # fp8 [128, 2, 240]
_CO_END = 2152


# ---------------------------------------------------------------- program ---

def build_program(repeat: int = 1):
    nc = bacc.Bacc("TRN2", target_bir_lowering=False, debug=False,
                   num_devices=N_CORES)

    d = {}
    def din(name, shape, dt):
        d[name] = nc.dram_tensor(name, shape, dt, kind="ExternalInput").ap()
        return d[name]

    x8_d = din("x8", [128, KT // 2, 2, N], FP8)        # xT fp8 ktile-pairs
    aqw8_d = din("aqw8", [128, HPC, KT // 2, 2, 128], FP8)  # fused a weights
    akw8_d = din("akw8", [128, HPC, KT // 2, 2, 224], FP8)  # fused c weights
    cst8_d = din("cst8", [128, _CO_END], U8)           # packed consts blob
    xT_d = din("xT", [DIM, N], BF16)                   # x bf16 (v path)
    wvT_d = din("wvT", [DIM, HPC * DH], BF16)          # v rhs (both heads)
    woutT_d = din("woutT", [DH, HPC, DIM], BF16)       # per-head Wout lhsT

    outT_d = nc.dram_tensor("outT", [DIM, N], BF16, kind="ExternalOutput").ap()

    with tile.TileContext(nc) as tc, ExitStack() as ctx:
        cst = ctx.enter_context(tc.tile_pool(name="cst", bufs=1))

        # --- loads; a/c path on the SP queue, the rest on the ACT queue ---
        x8t = cst.tile([128, KT // 2, 2, N], FP8, tag="x8")
        nc.sync.dma_start(x8t[:], x8_d[:])
        x8p = [x8t[:, pp] for pp in range(KT // 2)]
        aqw_t = cst.tile([128, HPC, KT // 2, 2, 128], FP8, tag="aqw8")
        nc.sync.dma_start(aqw_t[:], aqw8_d[:])
        aqw8 = [aqw_t[:, h] for h in range(HPC)]
        akw_t = cst.tile([128, HPC, KT // 2, 2, 224], FP8, tag="akw8")
        nc.sync.dma_start(akw_t[:], akw8_d[:])
        akw8 = [akw_t[:, h] for h in range(HPC)]
        consts = cst.tile([128, _CO_END], U8, tag="cst8")
        nc.scalar.dma_start(consts[:], cst8_d[:])
        s1c = consts[:, _CO_S1C:_CO_S1C + 4].bitcast(F32)
        b2r = consts[:, _CO_B2R:_CO_B2R + 4].bitcast(F32)
        w2b = consts[:, _CO_W2B:_CO_W2B + 128].bitcast(BF16)
        iden = consts[:, _CO_IDEN:_CO_IDEN + 256].bitcast(BF16)
        tri = consts[:, _CO_TRI:_CO_TRI + 1024].bitcast(BF16)
        w2b8 = consts[:, _CO_W2B8:_CO_W2B8 + 256].bitcast(FP8).rearrange(
            "p (i m) -> p i m", i=2)
        w3dr = consts[:, _CO_W3DR:_CO_W3DR + 480].bitcast(FP8).rearrange(
            "p (i m) -> p i m", i=2)
        warm = cst.tile([1, 4], F32, tag="warm")
        nc.vector.memset(warm[:], 0.0)
        nc.scalar.activation(warm[:], warm[:], AF.Exp)
        x_big = cst.tile([128, KT * N], BF16, tag="xT16")
        xv3 = xT_d.rearrange("(c a p) n -> p c a n", p=128, c=4)
        nc.scalar.dma_start(
            x_big[:].rearrange("p (c a n) -> p c a n", c=4, a=2), xv3)
        wv_big = cst.tile([128, KT * HPC * DH], BF16, tag="wv")
        nc.scalar.dma_start(wv_big[:],
                            wvT_d.rearrange("(a p) m -> p a m", p=128))
        xT16 = [x_big[:, kk * N:(kk + 1) * N] for kk in range(KT)]
        wv = [wv_big[:, kk * HPC * DH:(kk + 1) * HPC * DH] for kk in range(KT)]
        wout_t = cst.tile([DH, HPC, DIM], BF16, tag="woutT")
        nc.scalar.dma_start(wout_t[:], woutT_d[:])
        woutT = [wout_t[:, h] for h in range(HPC)]

        for rep in range(repeat):
            _body(nc, tc, ctx, rep, x8p, aqw8, akw8, xT16, wv, s1c, b2r,
                  w2b, w2b8, w3dr, tri, iden, woutT, outT_d)

    nc.compile()
    return nc


def _body(nc, tc, ctx, rep, x8p, aqw8, akw8, xT16, wv, s1c, b2r,
          w2b, w2b8, w3dr, tri, iden, woutT, outT_d):
    r = f"r{rep}"
    cst2 = ctx.enter_context(tc.tile_pool(name=f"cst2_{r}", bufs=1))

    ctr = {"s1f": 0, "s1b": 0, "s25": 0, "s25l": 0,
           "pair": TUNE["pair_ph"], "pairl": 0}

    def rot(key):
        pat = TUNE[key + "_pat"]
        c = ctr[key]
        ctr[key] += 1
        return pat[c % len(pat)]

    def pair_fp8(width, t):
        if t >= 2:
            # late tiles: separate dial; GPSIMD is otherwise idle by then
            if t > 2 or width < TUNE["fp8l_wmin"] or not TUNE["fp8l_num"]:
                return False
            c = ctr["pairl"]
            ctr["pairl"] += 1
            return (c * TUNE["fp8l_num"]) % TUNE["fp8l_den"] < TUNE["fp8l_num"]
        if width < TUNE["fp8_wmin"]:
            return False
        c = ctr["pair"]
        ctr["pair"] += 1
        return (c * TUNE["fp8_num"]) % TUNE["fp8_den"] < TUNE["fp8_num"]

    def ew(engine_ch, out_ap, in_ap, scal_ap):
        """relu(in + scal) on the chosen engine."""
        if engine_ch == "a":
            nc.scalar.activation(out_ap, in_ap, AF.Relu,
                                 bias=scal_ap, scale=1.0)
        else:
            e = nc.vector if engine_ch == "d" else nc.gpsimd
            e.tensor_scalar(out_ap, in_ap, scal_ap, 0.0, ALU.add, ALU.max)

    out_h = []  # [64, N] bf16 normalized attention output per head
    with tc.tile_pool(name=f"s2_{r}", bufs=TUNE["s2_bufs"], space="PSUM") as s2ps, \
         tc.tile_pool(name=f"sc_{r}", bufs=TUNE["sc_bufs"], space="PSUM") as scps, \
         tc.tile_pool(name=f"op_{r}", bufs=TUNE["op_bufs"], space="PSUM") as ops, \
         tc.tile_pool(name=f"wk_{r}", bufs=TUNE["h1_bufs"]) as wk, \
         tc.tile_pool(name=f"h2_{r}", bufs=TUNE["h2_bufs"]) as h2p, \
         tc.tile_pool(name=f"ex_{r}", bufs=TUNE["ex_bufs"]) as exp_pool:

        # c-path first for BOTH heads, then a-path: akw8 arrives first on
        # the sync queue, and this order avoids PE head-of-line blocking
        # (a ready-to-run a_ps would otherwise sit behind a c_ps whose
        # weights are still in flight).
        a4s, cbs, op_pss = [], [], []
        for h in range(HPC):
            # cb[32u+p, g] = S1*c[p, 4g+u]: sliding zero-padded window puts
            # block u at out rows [32u, 32u+32); rhs is x columns 4g+u.
            c_ps = scps.tile([128, 128], F32, tag="sc")
            for u in range(4):
                s0 = 96 - 32 * u
                for pp in range(KT // 2):
                    nc.tensor.matmul(
                        c_ps[:, :], akw8[h][:, pp, :, s0:s0 + 128],
                        x8p[pp].rearrange("p i (g u) -> p i u g", u=4)[:, :, u],
                        start=(u == 0 and pp == 0),
                        stop=(u == 3 and pp == KT // 2 - 1),
                        perf_mode=PM.DoubleRow, skip_group_check=True)
            cb = cst2.tile([128, 128], F32, tag=f"cb_{h}")
            nc.vector.tensor_copy(cb[:], c_ps[:])
            cbs.append(cb)
        for h in range(HPC):
            # a4[32u+p, i] = S1*a[p, i] (replicated 4x) + S1*s1const
            a_ps = scps.tile([128, N], F32, tag="sc")
            for ch in range(2):
                c0, c1 = ch * (N // 2), (ch + 1) * (N // 2)
                for pp in range(KT // 2):
                    nc.tensor.matmul(a_ps[:, c0:c1], aqw8[h][:, pp],
                                     x8p[pp][:, :, c0:c1],
                                     start=(pp == 0), stop=(pp == KT // 2 - 1),
                                     perf_mode=PM.DoubleRow)
            a4 = cst2.tile([128, N], BF16, tag=f"a4_{h}")
            nc.vector.tensor_scalar(a4[:], a_ps[:], s1c, None, ALU.add)
            a4s.append(a4)

            # out' accumulator [65, N] psum (num rows 0..64, den row 64)
            op_ps = ops.tile([65, N], F32, tag=f"op{h}")
            op_pss.append(op_ps)

        # ---- v projection -> v' [128, 130] bf16 per j-tile (deferred; uses
        # an s2 pool slot so it fills PE gaps during early scoring).
        vp = cst2.tile([128, NT, 130], BF16, tag="vp")
        for t in TUNE["torder"]:
            ps_v = s2ps.tile([128, HPC * DH], F32, tag="s2")
            for kk in range(KT):
                nc.tensor.matmul(ps_v[:, :],
                                 xT16[kk][:, t * 128:(t + 1) * 128],
                                 wv[kk][:, :],
                                 start=(kk == 0), stop=(kk == KT - 1))
            for h in range(HPC):
                nc.vector.tensor_copy(vp[:, t, 65 * h:65 * h + DH],
                                      ps_v[:, h * DH:(h + 1) * DH])
                nc.vector.memset(vp[:, t, 65 * h + DH:65 * h + 65], 1.0)

        t_first, t_last = TUNE["torder"][0], TUNE["torder"][-1]
        for t in TUNE["torder"]:
            for h in range(HPC):
                a4, cb, op_ps = a4s[h], cbs[h], op_pss[h]
                L = N - t * 128
                i0 = t * 128
                sc_ps = scps.tile([128, L], F32, tag="sc")
                # init scores with causal mask (-1e30 in the i<j triangle,
                # 0 elsewhere); stage-3 matmuls then accumulate on top.
                nc.tensor.matmul(sc_ps[:, :], iden, tri[:, 0:L],
                                 start=True, stop=False,
                                 skip_group_check=True)
                for m0 in range(0, 16, 2):
                    ofs = 8 * m0
                    W = L - ofs
                    h1t = []
                    for dm in range(2):
                        m = m0 + dm
                        use8 = pair_fp8(W, t)
                        h1 = wk.tile([128, 2, W], FP8 if use8 else BF16,
                                     tag="h1")
                        ch = rot("s1f" if use8 else "s1b")  # per pair
                        for v in range(2):
                            g = 32 * t + 2 * m + v
                            ew(ch, h1[:, v, :], a4[:, i0 + ofs:N],
                               cb[:, g:g + 1])
                        h1t.append((h1, use8))
                    nch = 1 if W <= TUNE["cmax"] else 2
                    for chk in range(nch):
                        c0 = (W * chk) // nch
                        c1 = (W * (chk + 1)) // nch
                        C = c1 - c0
                        ps2 = s2ps.tile([128, 2, C], F32, tag="s2")
                        for dm in range(2):
                            h1, use8 = h1t[dm]
                            if use8:
                                nc.tensor.matmul(
                                    ps2[:, dm, :], w2b8,
                                    h1[:, :, c0:c1], start=True, stop=True,
                                    perf_mode=PM.DoubleRow)
                            else:
                                for v in range(2):
                                    nc.tensor.matmul(
                                        ps2[64 * v:64 * (v + 1), dm, :],
                                        w2b, h1[:, v, c0:c1],
                                        start=True, stop=True)
                        h2 = h2p.tile([128, 2, C], FP8, tag="h2")
                        ew(rot("s25" if t < 2 else "s25l"),
                           h2[:, :, :], ps2[:, :, :], b2r)
                        s0 = 112 - ofs
                        nc.tensor.matmul(
                            sc_ps[:, ofs + c0:ofs + c1],
                            w3dr[:, :, s0:s0 + 128], h2[:, :, :],
                            start=False,
                            stop=(m0 == 14 and chk == nch - 1),
                            perf_mode=PM.DoubleRow,
                            skip_group_check=True)
                ex = exp_pool.tile([128, L], BF16, tag="ex")
                nc.scalar.activation(ex[:], sc_ps[:], AF.Exp,
                                     scale=1.0 / (S1 * S2 * S3))
                nc.tensor.matmul(op_ps[:, i0:N],
                                 vp[:, t, 65 * h:65 * h + 65],
                                 ex[:], start=(t == t_first), stop=(t == t_last),
                                 skip_group_check=True)

                if t != t_last:
                    continue
                # last tile of this head: normalize (overlaps the other
                # head's t=0 tile).  out = num * (1/den), 1/den broadcast
                # via a K=1 ones matmul.
                rsb = cst2.tile([128, N], F32, tag=f"rec_{h}")
                nc.vector.reciprocal(rsb[64:65, :], op_ps[64:65, :])
                ones = cst2.tile([128, DH], F32, tag=f"ones_{h}")
                nc.vector.memset(ones[64:65, :], 1.0)
                rb_ps = scps.tile([DH, N], F32, tag="sc")
                nc.tensor.matmul(rb_ps[:, :], ones[64:65, :], rsb[64:65, :],
                                 start=True, stop=True)
                rb_sb = cst2.tile([DH, N], F32, tag=f"rbs_{h}")
                nc.vector.tensor_copy(rb_sb[:], rb_ps[:])
                o = cst2.tile([DH, N], BF16, tag=f"out_{h}")
                nc.vector.tensor_tensor(o[:], op_ps[0:DH, :], rb_sb[:],
                                        ALU.mult)
                out_h.append(o)

    # ---------------- P4: output projection (row-parallel Wout) ------------
    with tc.tile_pool(name=f"wo_{r}", bufs=4, space="PSUM") as wops, \
         tc.tile_pool(name=f"ob_{r}", bufs=8) as obp:
        for ot in range(KT):
            ps = wops.tile([128, N], F32, tag="wo")
            for h in range(HPC):
                nc.tensor.matmul(ps[:, :],
                                 woutT[h][:, ot * 128:(ot + 1) * 128],
                                 out_h[h][:, :],
                                 start=(h == 0), stop=(h == HPC - 1))
            ob = obp.tile([128, N], BF16, tag="ob")
            if ot % 2 == 0:
                nc.vector.tensor_copy(ob[:], ps[:])
            else:
                nc.scalar.copy(ob[:], ps[:])
            nc.sync.dma_start(
                outT_d.rearrange("(a p) n -> a p n", p=128)[ot], ob[:])


# ---------------------------------------------------------------- host side -

def prep_inputs(x, Wqkv, Wout, Wq, bq, Wk, bk, W1, b1, W2, b2, W3, b3):
    """Build the per-core input maps (all numpy)."""
    x = np.asarray(x, np.float32).reshape(N, DIM)
    Wqkv = np.asarray(Wqkv, np.float32)
    Wout = np.asarray(Wout, np.float32)
    Wq, bq = np.asarray(Wq, np.float32), np.asarray(bq, np.float32)
    Wk, bk = np.asarray(Wk, np.float32), np.asarray(bk, np.float32)
    W1, b1 = np.asarray(W1, np.float32), np.asarray(b1, np.float32)
    W2, b2 = np.asarray(W2, np.float32), np.asarray(b2, np.float32)
    W3 = np.asarray(W3, np.float32)

    bf = lambda a: np.ascontiguousarray(a).astype(ml_dtypes.bfloat16)
    f8 = lambda a: np.ascontiguousarray(a).astype(ml_dtypes.float8_e4m3)
    u8 = lambda a: np.ascontiguousarray(a).view(np.uint8)

    xT = x.T                                        # [DIM, N] f32
    x8p = f8(xT.reshape(4, 2, 128, N).transpose(2, 0, 1, 3))

    W1q, W1k = W1[:, :P], W1[:, P:]
    AqP = S1 * (W1q @ Wq)                           # [32, 64]
    AkP = S1 * (W1k @ Wk)
    s1const = S1 * (W1q @ bq + W1k @ bk + b1)       # [32]

    # ---- packed constants blob ----
    s1c = np.tile(s1const, 4)[:, None].astype(np.float32)      # [128,1]
    b2r = np.tile(S1 * S2 * b2, 8)[:, None].astype(np.float32)
    w2b = np.zeros((128, 64), np.float32)
    for u in range(4):
        w2b[32 * u:32 * (u + 1), 16 * u:16 * (u + 1)] = S2 * W2.T
    iden = np.eye(128, dtype=np.float32)
    ii = np.arange(128)
    tri = np.zeros((128, N), np.float32)        # [j, i]: 0 valid, -1e30 not
    tri[:, 0:128] = np.where(ii[None, :] >= ii[:, None], 0.0, -1e30)
    w2b8 = np.zeros((128, 2, 128), np.float32)
    w2b8[:, 0, 0:64] = w2b
    w2b8[:, 1, 64:128] = w2b
    w3dr = np.zeros((128, 2, 240), np.float32)
    for dm in range(2):
        for v in range(2):
            for u in range(4):
                col = 112 + 8 * dm + 4 * v + u
                for q in range(P2):
                    w3dr[64 * v + 16 * u + q, dm, col] = S3 * W3[0, q]
    blob = np.concatenate([
        u8(s1c), u8(b2r), u8(bf(w2b)), u8(bf(iden)), u8(bf(tri)),
        u8(f8(w2b8).reshape(128, 256)), u8(f8(w3dr).reshape(128, 480)),
    ], axis=1)
    assert blob.shape == (128, _CO_END), blob.shape

    # per-head channel index in Wqkv output: o = d*48 + k*16 + h
    dch = np.arange(DH)
    in_maps = []
    for c in range(N_CORES):
        heads = (HPC * c, HPC * c + 1)
        aqw8 = np.zeros((128, HPC, 4, 2, 128), np.float32)
        akw8 = np.zeros((128, HPC, 4, 2, 224), np.float32)
        for hh, h in enumerate(heads):
            Wqh = Wqkv[dch * 48 + 0 * HEADS + h]     # [64, DIM]
            Wkh = Wqkv[dch * 48 + 1 * HEADS + h]
            AqW = AqP @ Wqh                          # [32, DIM]
            AkW = AkP @ Wkh
            # a weights: out row 32u+p = AqW[p]; [DIM,128] -> ktile pairs
            aw = np.tile(AqW, (4, 1)).T              # [DIM, 128]
            aqw8[:, hh] = aw.reshape(4, 2, 128, 128).transpose(2, 0, 1, 3)
            # c weights: sliding window, live cols [96, 128) hold AkW
            kw = np.zeros((DIM, 224), np.float32)
            kw[:, 96:128] = AkW.T
            akw8[:, hh] = kw.reshape(4, 2, 128, 224).transpose(2, 0, 1, 3)
        rows_v = [dch * 48 + 2 * HEADS + h for h in heads]
        wvT = np.concatenate([Wqkv[r] for r in rows_v], axis=0).T  # [DIM,128]
        woutT = np.stack(
            [Wout[:, DH * h:DH * (h + 1)].T for h in heads],
            axis=1)                                      # [64, 2, DIM]
        in_maps.append({
            "x8": x8p,
            "aqw8": f8(aqw8),
            "akw8": f8(akw8),
            "cst8": blob,
            "xT": bf(xT),
            "wvT": bf(wvT),
            "woutT": bf(woutT),
        })
    return in_maps


_PROGRAM_CACHE = {}


def _get_program(repeat=1):
    if repeat not in _PROGRAM_CACHE:
        _PROGRAM_CACHE[repeat] = build_program(repeat)
    return _PROGRAM_CACHE[repeat]


def run(in_maps, repeat=1):
    nc = _get_program(repeat)
    return run_bass_kernel_spmd(nc, in_maps, list(range(N_CORES)))


def kernel(**inputs) -> np.ndarray:
    in_maps = prep_inputs(**inputs)
    res = run(in_maps)
    acc = np.zeros((DIM, N), np.float64)
    for c in range(N_CORES):
        acc += res.results[c]["outT"].astype(np.float64)
    return np.ascontiguousarray(acc.T.astype(np.float32)).reshape(B, N, DIM)


# revision 73
# speedup vs baseline: 1.0430x; 1.0430x over previous
"""Trainium2 Bass kernel for nn_NeuralAttention (MLP-scored attention).

Math (per head h, batch 1, n=512, dh=64, P=32):
  qkv = x @ Wqkv^T, split 'b n (d k h) -> k b h n d'
  qp = q@Wq^T+bq ; kp = k@Wk^T+bk
  a  = qp@W1q^T  ; c = kp@W1k^T          (W1 = [W1q | W1k])
  h1 = relu(a_i + c_j + b1)              # [n, n, 32]
  h2 = relu(h1 @ W2^T + b2)              # [n, n, 16]
  s  = h2 @ W3^T (+ b3, drops in softmax)
  attn = softmax(causal(s)) ; out = attn @ v ; y = out @ Wout^T

The scores are numerically near-uniform (std ~1e-4), so the whole
score path tolerates fp8.  The q/k projections are folded into the
a/c weights (a = x @ (W1q Wq Wqkv_q)^T) and computed as fp8
DoubleRow matmuls (K=256, 0.5 cyc/col), as are stage-2 (for a
tunable fraction of pairs) and stage-3.  The v / attn@v / Wout path
stays bf16 (errors there hit the output directly).  Scaling: h1 x16
(folded into a/c weights + s1const), W2 x4 (h2 x64), W3 x16; the
exp un-scales by 1/1024 via the activation scale argument.

Sharding: 16 heads over 8 cores (2 heads/core), Wout row-parallel;
host sums the 8 partial bf16 [1024, 512] outputs and transposes.

Layout ("j on partitions"): scores^T[j, i] in j-tiles of 128; pair
groups (m0, m0+1) share causal column offset 8*m0.  Stage-1
relu(a_i + c_j) is spread over DVE (bf16 4x / fp8 2x), GPSIMD and
ACT; stage-2.5 relu -> fp8 h2 over ACT/DVE (GPSIMD cannot read
PSUM).  DoubleRow dst must start at partition 0, so stage-3 (and
the c-bias matmul) use sliding zero-padded weight windows that park
each group's 16 (or 32) live rows at the right output partitions.
Scores PSUM is pre-initialized by an identity matmul with the
causal -1e30 mask; softmax denominator comes free from a
ones-column in the attn@v matmul.

Modeled (TimelineSim) per-core time: ~68.1 us (baseline 77.7);
measured rel. error vs fp32 reference: 4.3e-3.
"""

import sys

sys.path.insert(0, "/opt/trn_rl_repo")

from contextlib import ExitStack

import ml_dtypes
import numpy as np

import concourse.bass as bass
import concourse.tile as tile
from concourse import bacc, mybir
from concourse.bass_utils import run_bass_kernel_spmd

F32 = mybir.dt.float32
BF16 = mybir.dt.bfloat16
FP8 = mybir.dt.float8e4
U8 = mybir.dt.uint8
AF = mybir.ActivationFunctionType
ALU = mybir.AluOpType
PM = mybir.MatmulPerfMode

B, N, DIM = 1, 512, 1024
HEADS, DH = 16, 64
P, P2 = 32, 16
N_CORES = 8
HPC = HEADS // N_CORES  # heads per core = 2

S1 = 16.0   # h1 scale (folded into a/c weights and s1const)
S2 = 4.0    # extra W2 scale (h2 scale = S1*S2 = 64)
S3 = 16.0   # W3 scale; total score scale S1*S2*S3 = 1024

# scheduling tunables
TUNE = dict(
    fp8_num=7, fp8_den=20,   # fraction of large pairs with fp8 stage-2
    fp8_wmin=200,            # pairs narrower than this stay bf16
    fp8l_num=0, fp8l_den=20, # same for t==2 tiles (0 = off)
    fp8l_wmin=100,
    pair_ph=0,               # fp8 Bresenham phase offset
    s1f_pat="p",             # engine rotation for fp8 stage-1 (d/p/a)
    s1b_pat="d",        # engine rotation for bf16 stage-1
    s25_pat="aaad",          # engine rotation for stage-2.5 (no "p": PSUM)
    s25l_pat="aada",           # stage-2.5 rotation for late (small) tiles
    cmax=256,                # psum column chunk for stage-2/3
    s2_bufs=4,               # stage-2 psum tiles
    h1_bufs=36,              # stage-1 sbuf tiles
    h2_bufs=12,              # stage-2.5 sbuf tiles
    ex_bufs=5,               # exp sbuf tiles
    sc_bufs=2,               # scores psum tiles (shared tag, both heads)
    torder=(0, 1, 2, 3),     # j-tile processing order
    op_bufs=1,               # out' accumulator psum tiles (per head tag)
)
NT = N // 128           # j tiles = 4
KT = DIM // 128         # contraction tiles for projections = 8

# byte offsets in the packed constants blob (all 4-byte aligned)
_CO_S1C = 0            # f32 [128, 1]
_CO_B2R = 4            # f32 [128, 1]
_CO_W2B = 8            # bf16 [128, 64]
_CO_IDEN = 136         # bf16 [128, 128]
_CO_TRI = 392          # bf16 [128, 512]
_CO_W2B8 = 1416        # fp8 [128, 2, 128]
_CO_W3DR = 1672        # fp8 [128, 2, 240]
_CO_END = 2152


# ---------------------------------------------------------------- program ---

def build_program(repeat: int = 1):
    nc = bacc.Bacc("TRN2", target_bir_lowering=False, debug=False,
                   num_devices=N_CORES)

    d = {}
    def din(name, shape, dt):
        d[name] = nc.dram_tensor(name, shape, dt, kind="ExternalInput").ap()
        return d[name]

    x8_d = din("x8", [128, KT // 2, 2, N], FP8)        # xT fp8 ktile-pairs
    aqw8_d = din("aqw8", [128, HPC, KT // 2, 2, 128], FP8)  # fused a weights
    akw8_d = din("akw8", [128, HPC, KT // 2, 2, 224], FP8)  # fused c weights
    cst8_d = din("cst8", [128, _CO_END], U8)           # packed consts blob
    xT_d = din("xT", [DIM, N], BF16)                   # x bf16 (v path)
    wvT_d = din("wvT", [DIM, HPC * DH], BF16)          # v rhs (both heads)
    woutT_d = din("woutT", [DH, HPC, DIM], BF16)       # per-head Wout lhsT

    outT_d = nc.dram_tensor("outT", [DIM, N], BF16, kind="ExternalOutput").ap()

    with tile.TileContext(nc) as tc, ExitStack() as ctx:
        cst = ctx.enter_context(tc.tile_pool(name="cst", bufs=1))

        # --- loads; a/c path on the SP queue, the rest on the ACT queue ---
        x8t = cst.tile([128, KT // 2, 2, N], FP8, tag="x8")
        nc.sync.dma_start(x8t[:], x8_d[:])
        x8p = [x8t[:, pp] for pp in range(KT // 2)]
        aqw_t = cst.tile([128, HPC, KT // 2, 2, 128], FP8, tag="aqw8")
        nc.sync.dma_start(aqw_t[:], aqw8_d[:])
        aqw8 = [aqw_t[:, h] for h in range(HPC)]
        akw_t = cst.tile([128, HPC, KT // 2, 2, 224], FP8, tag="akw8")
        nc.sync.dma_start(akw_t[:], akw8_d[:])
        akw8 = [akw_t[:, h] for h in range(HPC)]
        consts = cst.tile([128, _CO_END], U8, tag="cst8")
        nc.scalar.dma_start(consts[:], cst8_d[:])
        s1c = consts[:, _CO_S1C:_CO_S1C + 4].bitcast(F32)
        b2r = consts[:, _CO_B2R:_CO_B2R + 4].bitcast(F32)
        w2b = consts[:, _CO_W2B:_CO_W2B + 128].bitcast(BF16)
        iden = consts[:, _CO_IDEN:_CO_IDEN + 256].bitcast(BF16)
        tri = consts[:, _CO_TRI:_CO_TRI + 1024].bitcast(BF16)
        w2b8 = consts[:, _CO_W2B8:_CO_W2B8 + 256].bitcast(FP8).rearrange(
            "p (i m) -> p i m", i=2)
        w3dr = consts[:, _CO_W3DR:_CO_W3DR + 480].bitcast(FP8).rearrange(
            "p (i m) -> p i m", i=2)
        warm = cst.tile([1, 4], F32, tag="warm")
        nc.vector.memset(warm[:], 0.0)
        nc.scalar.activation(warm[:], warm[:], AF.Exp)
        x_big = cst.tile([128, KT * N], BF16, tag="xT16")
        xv3 = xT_d.rearrange("(c a p) n -> p c a n", p=128, c=4)
        nc.scalar.dma_start(
            x_big[:].rearrange("p (c a n) -> p c a n", c=4, a=2), xv3)
        wv_big = cst.tile([128, KT * HPC * DH], BF16, tag="wv")
        nc.scalar.dma_start(wv_big[:],
                            wvT_d.rearrange("(a p) m -> p a m", p=128))
        xT16 = [x_big[:, kk * N:(kk + 1) * N] for kk in range(KT)]
        wv = [wv_big[:, kk * HPC * DH:(kk + 1) * HPC * DH] for kk in range(KT)]
        wout_t = cst.tile([DH, HPC, DIM], BF16, tag="woutT")
        nc.scalar.dma_start(wout_t[:], woutT_d[:])
        woutT = [wout_t[:, h] for h in range(HPC)]

        for rep in range(repeat):
            _body(nc, tc, ctx, rep, x8p, aqw8, akw8, xT16, wv, s1c, b2r,
                  w2b, w2b8, w3dr, tri, iden, woutT, outT_d)

    nc.compile()
    return nc


def _body(nc, tc, ctx, rep, x8p, aqw8, akw8, xT16, wv, s1c, b2r,
          w2b, w2b8, w3dr, tri, iden, woutT, outT_d):
    r = f"r{rep}"
    cst2 = ctx.enter_context(tc.tile_pool(name=f"cst2_{r}", bufs=1))

    ctr = {"s1f": 0, "s1b": 0, "s25": 0, "s25l": 0,
           "pair": TUNE["pair_ph"], "pairl": 0}

    def rot(key):
        pat = TUNE[key + "_pat"]
        c = ctr[key]
        ctr[key] += 1
        return pat[c % len(pat)]

    def pair_fp8(width, t):
        if t >= 2:
            # late tiles: separate dial; GPSIMD is otherwise idle by then
            if t > 2 or width < TUNE["fp8l_wmin"] or not TUNE["fp8l_num"]:
                return False
            c = ctr["pairl"]
            ctr["pairl"] += 1
            return (c * TUNE["fp8l_num"]) % TUNE["fp8l_den"] < TUNE["fp8l_num"]
        if width < TUNE["fp8_wmin"]:
            return False
        c = ctr["pair"]
        ctr["pair"] += 1
        return (c * TUNE["fp8_num"]) % TUNE["fp8_den"] < TUNE["fp8_num"]

    def ew(engine_ch, out_ap, in_ap, scal_ap):
        """relu(in + scal) on the chosen engine."""
        if engine_ch == "a":
            nc.scalar.activation(out_ap, in_ap, AF.Relu,
                                 bias=scal_ap, scale=1.0)
        else:
            e = nc.vector if engine_ch == "d" else nc.gpsimd
            e.tensor_scalar(out_ap, in_ap, scal_ap, 0.0, ALU.add, ALU.max)

    out_h = []  # [64, N] bf16 normalized attention output per head
    with tc.tile_pool(name=f"s2_{r}", bufs=TUNE["s2_bufs"], space="PSUM") as s2ps, \
         tc.tile_pool(name=f"sc_{r}", bufs=TUNE["sc_bufs"], space="PSUM") as scps, \
         tc.tile_pool(name=f"op_{r}", bufs=TUNE["op_bufs"], space="PSUM") as ops, \
         tc.tile_pool(name=f"wk_{r}", bufs=TUNE["h1_bufs"]) as wk, \
         tc.tile_pool(name=f"h2_{r}", bufs=TUNE["h2_bufs"]) as h2p, \
         tc.tile_pool(name=f"ex_{r}", bufs=TUNE["ex_bufs"]) as exp_pool:

        a4s, cbs, op_pss = [], [], []
        for h in range(HPC):
            # a4[32u+p, i] = S1*a[p, i] (replicated 4x) + S1*s1const
            a_ps = scps.tile([128, N], F32, tag="sc")
            for ch in range(2):
                c0, c1 = ch * (N // 2), (ch + 1) * (N // 2)
                for pp in range(KT // 2):
                    nc.tensor.matmul(a_ps[:, c0:c1], aqw8[h][:, pp],
                                     x8p[pp][:, :, c0:c1],
                                     start=(pp == 0), stop=(pp == KT // 2 - 1),
                                     perf_mode=PM.DoubleRow)
            a4 = cst2.tile([128, N], BF16, tag=f"a4_{h}")
            nc.vector.tensor_scalar(a4[:], a_ps[:], s1c, None, ALU.add)
            a4s.append(a4)

            # cb[32u+p, g] = S1*c[p, 4g+u]: sliding zero-padded window puts
            # block u at out rows [32u, 32u+32); rhs is x columns 4g+u.
            c_ps = scps.tile([128, 128], F32, tag="sc")
            for u in range(4):
                s0 = 96 - 32 * u
                for pp in range(KT // 2):
                    nc.tensor.matmul(
                        c_ps[:, :], akw8[h][:, pp, :, s0:s0 + 128],
                        x8p[pp].rearrange("p i (g u) -> p i u g", u=4)[:, :, u],
                        start=(u == 0 and pp == 0),
                        stop=(u == 3 and pp == KT // 2 - 1),
                        perf_mode=PM.DoubleRow, skip_group_check=True)
            cb = cst2.tile([128, 128], F32, tag=f"cb_{h}")
            nc.vector.tensor_copy(cb[:], c_ps[:])
            cbs.append(cb)

            # out' accumulator [65, N] psum (num rows 0..64, den row 64)
            op_ps = ops.tile([65, N], F32, tag=f"op{h}")
            op_pss.append(op_ps)

        # ---- v projection -> v' [128, 130] bf16 per j-tile (deferred; uses
        # an s2 pool slot so it fills PE gaps during early scoring).
        vp = cst2.tile([128, NT, 130], BF16, tag="vp")
        for t in TUNE["torder"]:
            ps_v = s2ps.tile([128, HPC * DH], F32, tag="s2")
            for kk in range(KT):
                nc.tensor.matmul(ps_v[:, :],
                                 xT16[kk][:, t * 128:(t + 1) * 128],
                                 wv[kk][:, :],
                                 start=(kk == 0), stop=(kk == KT - 1))
            for h in range(HPC):
                nc.vector.tensor_copy(vp[:, t, 65 * h:65 * h + DH],
                                      ps_v[:, h * DH:(h + 1) * DH])
                nc.vector.memset(vp[:, t, 65 * h + DH:65 * h + 65], 1.0)

        t_first, t_last = TUNE["torder"][0], TUNE["torder"][-1]
        for t in TUNE["torder"]:
            for h in range(HPC):
                a4, cb, op_ps = a4s[h], cbs[h], op_pss[h]
                L = N - t * 128
                i0 = t * 128
                sc_ps = scps.tile([128, L], F32, tag="sc")
                # init scores with causal mask (-1e30 in the i<j triangle,
                # 0 elsewhere); stage-3 matmuls then accumulate on top.
                nc.tensor.matmul(sc_ps[:, :], iden, tri[:, 0:L],
                                 start=True, stop=False,
                                 skip_group_check=True)
                for m0 in range(0, 16, 2):
                    ofs = 8 * m0
                    W = L - ofs
                    h1t = []
                    for dm in range(2):
                        m = m0 + dm
                        use8 = pair_fp8(W, t)
                        h1 = wk.tile([128, 2, W], FP8 if use8 else BF16,
                                     tag="h1")
                        ch = rot("s1f" if use8 else "s1b")  # per pair
                        for v in range(2):
                            g = 32 * t + 2 * m + v
                            ew(ch, h1[:, v, :], a4[:, i0 + ofs:N],
                               cb[:, g:g + 1])
                        h1t.append((h1, use8))
                    nch = 1 if W <= TUNE["cmax"] else 2
                    for chk in range(nch):
                        c0 = (W * chk) // nch
                        c1 = (W * (chk + 1)) // nch
                        C = c1 - c0
                        ps2 = s2ps.tile([128, 2, C], F32, tag="s2")
                        for dm in range(2):
                            h1, use8 = h1t[dm]
                            if use8:
                                nc.tensor.matmul(
                                    ps2[:, dm, :], w2b8,
                                    h1[:, :, c0:c1], start=True, stop=True,
                                    perf_mode=PM.DoubleRow)
                            else:
                                for v in range(2):
                                    nc.tensor.matmul(
                                        ps2[64 * v:64 * (v + 1), dm, :],
                                        w2b, h1[:, v, c0:c1],
                                        start=True, stop=True)
                        h2 = h2p.tile([128, 2, C], FP8, tag="h2")
                        ew(rot("s25" if t < 2 else "s25l"),
                           h2[:, :, :], ps2[:, :, :], b2r)
                        s0 = 112 - ofs
                        nc.tensor.matmul(
                            sc_ps[:, ofs + c0:ofs + c1],
                            w3dr[:, :, s0:s0 + 128], h2[:, :, :],
                            start=False,
                            stop=(m0 == 14 and chk == nch - 1),
                            perf_mode=PM.DoubleRow,
                            skip_group_check=True)
                ex = exp_pool.tile([128, L], BF16, tag="ex")
                nc.scalar.activation(ex[:], sc_ps[:], AF.Exp,
                                     scale=1.0 / (S1 * S2 * S3))
                nc.tensor.matmul(op_ps[:, i0:N],
                                 vp[:, t, 65 * h:65 * h + 65],
                                 ex[:], start=(t == t_first), stop=(t == t_last),
                                 skip_group_check=True)

                if t != t_last:
                    continue
                # last tile of this head: normalize (overlaps the other
                # head's t=0 tile).  out = num * (1/den), 1/den broadcast
                # via a K=1 ones matmul.
                rsb = cst2.tile([128, N], F32, tag=f"rec_{h}")
                nc.vector.reciprocal(rsb[64:65, :], op_ps[64:65, :])
                ones = cst2.tile([128, DH], F32, tag=f"ones_{h}")
                nc.vector.memset(ones[64:65, :], 1.0)
                rb_ps = scps.tile([DH, N], F32, tag="sc")
                nc.tensor.matmul(rb_ps[:, :], ones[64:65, :], rsb[64:65, :],
                                 start=True, stop=True)
                rb_sb = cst2.tile([DH, N], F32, tag=f"rbs_{h}")
                nc.vector.tensor_copy(rb_sb[:], rb_ps[:])
                o = cst2.tile([DH, N], BF16, tag=f"out_{h}")
                nc.vector.tensor_tensor(o[:], op_ps[0:DH, :], rb_sb[:],
                                        ALU.mult)
                out_h.append(o)

    # ---------------- P4: output projection (row-parallel Wout) ------------
    with tc.tile_pool(name=f"wo_{r}", bufs=4, space="PSUM") as wops, \
         tc.tile_pool(name=f"ob_{r}", bufs=8) as obp:
        for ot in range(KT):
            ps = wops.tile([128, N], F32, tag="wo")
            for h in range(HPC):
                nc.tensor.matmul(ps[:, :],
                                 woutT[h][:, ot * 128:(ot + 1) * 128],
                                 out_h[h][:, :],
                                 start=(h == 0), stop=(h == HPC - 1))
            ob = obp.tile([128, N], BF16, tag="ob")
            if ot % 2 == 0:
                nc.vector.tensor_copy(ob[:], ps[:])
            else:
                nc.scalar.copy(ob[:], ps[:])
            nc.sync.dma_start(
                outT_d.rearrange("(a p) n -> a p n", p=128)[ot], ob[:])


# ---------------------------------------------------------------- host side -

def prep_inputs(x, Wqkv, Wout, Wq, bq, Wk, bk, W1, b1, W2, b2, W3, b3):
    """Build the per-core input maps (all numpy)."""
    x = np.asarray(x, np.float32).reshape(N, DIM)
    Wqkv = np.asarray(Wqkv, np.float32)
    Wout = np.asarray(Wout, np.float32)
    Wq, bq = np.asarray(Wq, np.float32), np.asarray(bq, np.float32)
    Wk, bk = np.asarray(Wk, np.float32), np.asarray(bk, np.float32)
    W1, b1 = np.asarray(W1, np.float32), np.asarray(b1, np.float32)
    W2, b2 = np.asarray(W2, np.float32), np.asarray(b2, np.float32)
    W3 = np.asarray(W3, np.float32)

    bf = lambda a: np.ascontiguousarray(a).astype(ml_dtypes.bfloat16)
    f8 = lambda a: np.ascontiguousarray(a).astype(ml_dtypes.float8_e4m3)
    u8 = lambda a: np.ascontiguousarray(a).view(np.uint8)

    xT = x.T                                        # [DIM, N] f32
    x8p = f8(xT.reshape(4, 2, 128, N).transpose(2, 0, 1, 3))

    W1q, W1k = W1[:, :P], W1[:, P:]
    AqP = S1 * (W1q @ Wq)                           # [32, 64]
    AkP = S1 * (W1k @ Wk)
    s1const = S1 * (W1q @ bq + W1k @ bk + b1)       # [32]

    # ---- packed constants blob ----
    s1c = np.tile(s1const, 4)[:, None].astype(np.float32)      # [128,1]
    b2r = np.tile(S1 * S2 * b2, 8)[:, None].astype(np.float32)
    w2b = np.zeros((128, 64), np.float32)
    for u in range(4):
        w2b[32 * u:32 * (u + 1), 16 * u:16 * (u + 1)] = S2 * W2.T
    iden = np.eye(128, dtype=np.float32)
    ii = np.arange(128)
    tri = np.zeros((128, N), np.float32)        # [j, i]: 0 valid, -1e30 not
    tri[:, 0:128] = np.where(ii[None, :] >= ii[:, None], 0.0, -1e30)
    w2b8 = np.zeros((128, 2, 128), np.float32)
    w2b8[:, 0, 0:64] = w2b
    w2b8[:, 1, 64:128] = w2b
    w3dr = np.zeros((128, 2, 240), np.float32)
    for dm in range(2):
        for v in range(2):
            for u in range(4):
                col = 112 + 8 * dm + 4 * v + u
                for q in range(P2):
                    w3dr[64 * v + 16 * u + q, dm, col] = S3 * W3[0, q]
    blob = np.concatenate([
        u8(s1c), u8(b2r), u8(bf(w2b)), u8(bf(iden)), u8(bf(tri)),
        u8(f8(w2b8).reshape(128, 256)), u8(f8(w3dr).reshape(128, 480)),
    ], axis=1)
    assert blob.shape == (128, _CO_END), blob.shape

    # per-head channel index in Wqkv output: o = d*48 + k*16 + h
    dch = np.arange(DH)
    in_maps = []
    for c in range(N_CORES):
        heads = (HPC * c, HPC * c + 1)
        aqw8 = np.zeros((128, HPC, 4, 2, 128), np.float32)
        akw8 = np.zeros((128, HPC, 4, 2, 224), np.float32)
        for hh, h in enumerate(heads):
            Wqh = Wqkv[dch * 48 + 0 * HEADS + h]     # [64, DIM]
            Wkh = Wqkv[dch * 48 + 1 * HEADS + h]
            AqW = AqP @ Wqh                          # [32, DIM]
            AkW = AkP @ Wkh
            # a weights: out row 32u+p = AqW[p]; [DIM,128] -> ktile pairs
            aw = np.tile(AqW, (4, 1)).T              # [DIM, 128]
            aqw8[:, hh] = aw.reshape(4, 2, 128, 128).transpose(2, 0, 1, 3)
            # c weights: sliding window, live cols [96, 128) hold AkW
            kw = np.zeros((DIM, 224), np.float32)
            kw[:, 96:128] = AkW.T
            akw8[:, hh] = kw.reshape(4, 2, 128, 224).transpose(2, 0, 1, 3)
        rows_v = [dch * 48 + 2 * HEADS + h for h in heads]
        wvT = np.concatenate([Wqkv[r] for r in rows_v], axis=0).T  # [DIM,128]
        woutT = np.stack(
            [Wout[:, DH * h:DH * (h + 1)].T for h in heads],
            axis=1)                                      # [64, 2, DIM]
        in_maps.append({
            "x8": x8p,
            "aqw8": f8(aqw8),
            "akw8": f8(akw8),
            "cst8": blob,
            "xT": bf(xT),
            "wvT": bf(wvT),
            "woutT": bf(woutT),
        })
    return in_maps


_PROGRAM_CACHE = {}


def _get_program(repeat=1):
    if repeat not in _PROGRAM_CACHE:
        _PROGRAM_CACHE[repeat] = build_program(repeat)
    return _PROGRAM_CACHE[repeat]


def run(in_maps, repeat=1):
    nc = _get_program(repeat)
    return run_bass_kernel_spmd(nc, in_maps, list(range(N_CORES)))


def kernel(**inputs) -> np.ndarray:
    in_maps = prep_inputs(**inputs)
    res = run(in_maps)
    acc = np.zeros((DIM, N), np.float64)
    for c in range(N_CORES):
        acc += res.results[c]["outT"].astype(np.float64)
    return np.ascontiguousarray(acc.T.astype(np.float32)).reshape(B, N, DIM)
